# revision 3
# baseline (speedup 1.0000x reference)
"""Trainium2 Bass kernel for nn_CrossSelfAttention (B=2, C=64, H=W=64, dk=8).

Mathematical structure exploited (guaranteed by the model's constructor,
asserted at runtime):
  * All Sobel conv weights are a single 3x3 kernel broadcast over every
    (out, in) channel pair, so conv3(x, W)[o] = K (*) sum_c x[c] for every o
    -- each conv collapses to one 2D conv on the channel-summed image.
  * Hence xq[m, d] = alpha_q[d] * Eq[m] + b1_q[d] (rank-1 in the spatial
    index), same for the keys, and the softmax logits reduce to
    S[m, n] = t[m] * Ek[n] + (terms constant in n, which cancel in softmax),
    with t[m] = (alpha_q . alpha_k) Eq[m] + (b1_q . alpha_k).

Per-core work (8 cores: (batch b, output modality, query-row half)):
  scores  S[n, m] = Ek[n] * t[m] - r[m] via K=12 bf16-split matmuls (PE);
          the operands are exact 3-way bf16 decompositions, so S is exact
          to ~1e-3 absolute on +-4000-magnitude logits.
  weights W = exp(S) (ACT, PSUM->SBUF, fp32r out, fused over two n-chunks)
  output  O = [V; 1]^T @ W accumulated over n (PE, fp32r), then divided by
          the ones-row (row sums) and DMA'd out.

r[m] = max(t*EkMax, t*EkMin) equals the true row max of S up to fp rounding;
any row offset cancels exactly in the normalization, so the softmax matches
the reference to ~2e-4 scaled error.
"""
import numpy as np
import ml_dtypes

_CACHE = {}

B, C, H, W = 2, 64, 64, 64
N = H * W          # 4096
MH = N // 2        # rows per core (query half)
NT = N // 128      # 32 n-chunks
MC = MH // 512     # 4 m-chunks per core


def _build_program():
    from contextlib import ExitStack
    import concourse.bass as bass
    import concourse.tile as tile
    from concourse import bacc, mybir

    f32 = mybir.dt.float32
    f32r = mybir.dt.float32r
    bf16 = mybir.dt.bfloat16
    Alu = mybir.AluOpType
    Act = mybir.ActivationFunctionType

    nc = bacc.Bacc("TRN2", num_devices=8)

    xa_d = nc.declare_dram_parameter("xaug", [C + 1, N], f32, isOutput=False)
    xk_d = nc.declare_dram_parameter("xkaug", [C + 1, N], f32, isOutput=False)
    wv_d = nc.declare_dram_parameter("wv_aug", [C + 1, C + 1], f32, isOutput=False)
    cs_d = nc.declare_dram_parameter("csum", [C + 1, 2], f32, isOutput=False)
    id_d = nc.declare_dram_parameter("iden", [C, C], f32, isOutput=False)
    cc_d = nc.declare_dram_parameter("cc", [C, 2], f32, isOutput=False)
    sel_d = nc.declare_dram_parameter("sel", [C, 32], f32, isOutput=False)
    kt_d = nc.declare_dram_parameter("ktap", [C, 18], f32, isOutput=False)
    o3_d = nc.declare_dram_parameter("ones3", [3, N], bf16, isOutput=False)
    o_d = nc.declare_dram_parameter("o", [C, MH], f32, isOutput=True)

    # DRAM scratch for layout bounces
    skr = nc.dram_tensor("skr", [N], f32)
    sqr = nc.dram_tensor("sqr", [N], f32)
    mm2 = nc.dram_tensor("mm2", [2], f32)
    ers = [nc.dram_tensor(f"er{i}", [N], bf16) for i in range(3)]
    trs = [nc.dram_tensor(f"tr{i}", [MH], bf16) for i in range(3)]
    rrs = [nc.dram_tensor(f"rr{i}", [MH], bf16) for i in range(3)]

    def bcast_ap(dram_handle, parts, count):
        base = dram_handle[:]
        return bass.AP(tensor=base.tensor, offset=base.offset,
                       ap=[[0, parts], [1, count]])

    with tile.TileContext(nc) as tc, ExitStack() as ctx:
        sb = ctx.enter_context(tc.tile_pool(name="sb", bufs=1))
        sbw = ctx.enter_context(tc.tile_pool(name="sbw", bufs=3))
        sbf = ctx.enter_context(tc.tile_pool(name="sbf", bufs=2))

        # ---------------- persistent SBUF ----------------
        xaug = sb.tile([C + 1, N], f32)
        xkaug = sb.tile([C + 1, N], f32)
        wv_aug = sb.tile([C + 1, C + 1], f32)
        csum = sb.tile([C + 1, 2], f32)
        iden = sb.tile([C, C], f32)
        cc = sb.tile([C, 2], f32)
        sel = sb.tile([C, 32], f32)
        ktap = sb.tile([C, 18], f32)
        nc.sync.dma_start(xaug[:], xa_d[:])
        nc.sync.dma_start(xkaug[:], xk_d[:])
        nc.sync.dma_start(wv_aug[:], wv_d[:])
        nc.sync.dma_start(csum[:], cs_d[:])
        nc.sync.dma_start(iden[:], id_d[:])
        nc.sync.dma_start(cc[:], cc_d[:])
        nc.sync.dma_start(sel[:], sel_d[:])
        nc.sync.dma_start(ktap[:], kt_d[:])

        vtr = sb.tile([128, NT * (C + 1)], f32r)     # [n, c+1] fp32r chunks
        s_v_col = sb.tile([128, NT], f32)
        s_k_col = sb.tile([128, NT], f32)
        s_q_col = sb.tile([128, NT], f32)
        esplit = sb.tile([12, N], bf16)
        tsplit = sb.tile([12, MH], bf16)
        emm = sb.tile([C, 2], f32)                   # EkMax / EkMin columns
        ones_row = sb.tile([1, C], f32)
        nc.vector.memset(ones_row[:], 1.0)
        nc.sync.dma_start(esplit[9:12, :], o3_d[:])

        # ---------------- setup phase ----------------
        with tc.tile_pool(name="psA", bufs=2, space="PSUM") as psA, \
             tc.tile_pool(name="psB", bufs=1, space="PSUM") as psB:

            # channel sums of both sources; one PSUM bank each, col per chunk
            psv = psB.tile([128, NT], f32, tag="psv")
            psk = psB.tile([128, NT], f32, tag="psk")
            for ch in range(NT):
                nc.tensor.matmul(psv[:, ch:ch + 1],
                                 xaug[:, ch * 128:(ch + 1) * 128],
                                 csum[:, 0:1], start=True, stop=True)
                nc.tensor.matmul(psk[:, ch:ch + 1],
                                 xkaug[:, ch * 128:(ch + 1) * 128],
                                 csum[:, 0:1], start=True, stop=True)
            nc.vector.tensor_copy(s_v_col[:], psv[:])
            nc.vector.tensor_copy(s_k_col[:], psk[:])
            nc.vector.tensor_add(s_q_col[:], s_v_col[:], s_k_col[:])

            # bounce col-layout sums (n = 128*j + p) to DRAM raster
            nc.sync.dma_start(
                skr.rearrange("(j p) -> p j", p=128)[:], s_k_col[:])
            nc.sync.dma_start(
                sqr.rearrange("(j p) -> p j", p=128)[:], s_q_col[:])

            # 3x3 SAME conv: pad_i[h, 1+w] = img[h+i-1, w] (zero border),
            # written by DMA so every compute AP starts at partition 0.
            def conv_abs2(raster, name):
                img2 = raster.rearrange("(h w) -> h w", h=H)
                pads = []
                for i in range(3):
                    pad = sb.tile([H, W + 2], f32, tag=f"pad{i}_{name}")
                    nc.vector.memset(pad[:], 0.0)
                    lo, hi = max(0, 1 - i), min(H, H + 1 - i)
                    nc.sync.dma_start(pad[lo:hi, 1:W + 1],
                                      img2[lo + i - 1:hi + i - 1, :])
                    pads.append(pad)
                outs = []
                for k0 in (0, 9):   # Kx taps cols 0..8, Ky taps cols 9..17
                    acc = sb.tile([H, W], f32, tag=f"acc{k0}_{name}")
                    nc.vector.tensor_scalar_mul(
                        acc[:], pads[0][0:H, 0:W], ktap[0:H, k0:k0 + 1])
                    for t9 in range(1, 9):
                        i, j = divmod(t9, 3)
                        nc.vector.scalar_tensor_tensor(
                            acc[:], pads[i][0:H, j:j + W],
                            ktap[0:H, k0 + t9:k0 + t9 + 1], acc[:],
                            op0=Alu.mult, op1=Alu.add)
                    neg = sb.tile([H, W], f32, tag=f"ng{k0}_{name}")
                    nc.vector.tensor_scalar_mul(neg[:], acc[:], -1.0)
                    aab = sb.tile([H, W], f32, tag=f"ab{k0}_{name}")
                    nc.vector.tensor_max(aab[:], acc[:], neg[:])
                    outs.append(aab)
                e_img = sb.tile([H, W], f32, tag=f"e_{name}")
                nc.vector.tensor_add(e_img[:], outs[0][:], outs[1][:])
                return e_img

            ek_img = conv_abs2(skr, "k")
            eq_img = conv_abs2(sqr, "q")

            # EkMax / EkMin scalars -> broadcast columns.
            # col1 carries -min so one 2-partition reduce_max covers both.
            mxmn = sb.tile([C, 2], f32)
            nc.vector.reduce_max(mxmn[:, 0:1], ek_img[:], axis=mybir.AxisListType.X)
            mnc = sb.tile([C, 1], f32)
            nc.vector.tensor_reduce(mnc[:], ek_img[:],
                                    axis=mybir.AxisListType.X, op=Alu.min)
            nc.vector.tensor_scalar_mul(mxmn[:, 1:2], mnc[:], -1.0)
            pmm = psB.tile([2, C], f32, tag="pmm")
            nc.tensor.transpose(pmm[:], mxmn[:], iden[:])
            sc2c = sb.tile([2, 1], f32)
            nc.vector.reduce_max(sc2c[:], pmm[:], axis=mybir.AxisListType.X)
            nc.sync.dma_start(mm2[None, :], sc2c.rearrange("p one -> one p")[:])
            nc.sync.dma_start(emm[:], bcast_ap(mm2, C, 2))
            nc.vector.tensor_scalar_mul(emm[:, 1:2], emm[:, 1:2], -1.0)

            # bf16 3-way split helper: x = s0 + s1 + s2 exactly (24 bits)
            def bsplit3(src, parts, name):
                sp = []
                cur = src
                for k in range(3):
                    bk = sb.tile([parts, src.shape[1]], bf16, tag=f"{name}b{k}")
                    nc.vector.tensor_copy(bk[:], cur[:])
                    sp.append(bk)
                    if k < 2:
                        bf = sb.tile([parts, src.shape[1]], f32, tag=f"{name}f{k}")
                        nc.vector.tensor_copy(bf[:], bk[:])
                        nxt = sb.tile([parts, src.shape[1]], f32, tag=f"{name}r{k}")
                        nc.vector.tensor_sub(nxt[:], cur[:], bf[:])
                        cur = nxt
                return sp

            # esplit rows: 3i+j = ek_i (flattened), rows 9..11 = 1.0
            eks = bsplit3(ek_img, H, "ek")
            for i in range(3):
                nc.sync.dma_start(
                    ers[i].rearrange("(h w) -> h w", h=H)[:], eks[i][:])
                nc.sync.dma_start(esplit[3 * i:3 * i + 3, :],
                                  bcast_ap(ers[i], 3, N))

            # Eq half via selection matmul, then t and r in [32, 64] layout
            pq = psB.tile([32, C], f32, tag="pq")
            nc.tensor.matmul(pq[:], sel[:], eq_img[:], start=True, stop=True)
            eqh = sb.tile([32, C], f32)
            nc.vector.tensor_copy(eqh[:], pq[:])
            t_img = sb.tile([32, C], f32)
            nc.vector.tensor_scalar(t_img[:], eqh[:], cc[0:32, 0:1],
                                    cc[0:32, 1:2], op0=Alu.mult, op1=Alu.add)
            a_img = sb.tile([32, C], f32)
            b_img = sb.tile([32, C], f32)
            nc.vector.tensor_scalar_mul(a_img[:], t_img[:], emm[0:32, 0:1])
            nc.vector.tensor_scalar_mul(b_img[:], t_img[:], emm[0:32, 1:2])
            r_img = sb.tile([32, C], f32)
            nc.vector.tensor_max(r_img[:], a_img[:], b_img[:])
            rn_img = sb.tile([32, C], f32)
            nc.vector.tensor_scalar_mul(rn_img[:], r_img[:], -1.0)

            # tsplit rows: 3i+j = t_j ; rows 9..11 = (-r)_j
            tjs = bsplit3(t_img, 32, "tj")
            rjs = bsplit3(rn_img, 32, "rj")
            for j in range(3):
                nc.sync.dma_start(
                    trs[j].rearrange("(h w) -> h w", h=32)[:], tjs[j][:])
                nc.sync.dma_start(
                    rrs[j].rearrange("(h w) -> h w", h=32)[:], rjs[j][:])
                for i in range(3):
                    k = 3 * i + j
                    nc.sync.dma_start(tsplit[k:k + 1, :], trs[j][None, :])
                nc.sync.dma_start(tsplit[9 + j:10 + j, :], rrs[j][None, :])

            # V matmul: VT chunks [128, C+1] -> fp32r (DVE convert-copy)
            for ch in range(NT):
                pv = psA.tile([128, C + 1], f32, tag="pv")
                nc.tensor.matmul(pv[:], xaug[:, ch * 128:(ch + 1) * 128],
                                 wv_aug[:], start=True, stop=True)
                nc.vector.tensor_copy(
                    vtr[:, ch * (C + 1):(ch + 1) * (C + 1)], pv[:])

        # ---------------- main loop ----------------
        with tc.tile_pool(name="psS", bufs=3, space="PSUM") as psS, \
             tc.tile_pool(name="psO", bufs=2, space="PSUM") as psO:
            for mc in range(MC):
                o_ps = psO.tile([C + 1, 512], mybir.dt.float32, tag="opsum")
                trh = tsplit[:, mc * 512:(mc + 1) * 512]
                for nt2 in range(NT // 2):
                    n0, n1 = 2 * nt2, 2 * nt2 + 1
                    s_ps = psS.tile([128, 1024], mybir.dt.float32, tag="spsum")
                    nc.tensor.matmul(s_ps[:, 0:512],
                                     esplit[:, n0 * 128:(n0 + 1) * 128],
                                     trh, start=True, stop=True)
                    nc.tensor.matmul(s_ps[:, 512:1024],
                                     esplit[:, n1 * 128:(n1 + 1) * 128],
                                     trh, start=True, stop=True)
                    wt = sbw.tile([128, 1024], f32r, tag="wt")
                    nc.scalar.activation(wt[:], s_ps[:], Act.Exp)
                    nc.tensor.matmul(
                        o_ps[:], vtr[:, n0 * (C + 1):(n0 + 1) * (C + 1)],
                        wt[:, 0:512], start=(nt2 == 0), stop=False)
                    nc.tensor.matmul(
                        o_ps[:], vtr[:, n1 * (C + 1):(n1 + 1) * (C + 1)],
                        wt[:, 512:1024], start=False, stop=(nt2 == NT // 2 - 1))

                rec = sbf.tile([1, 512], f32, tag="rec")
                nc.vector.reciprocal(rec[:], o_ps[C:C + 1, :])
                pb = psS.tile([C, 512], mybir.dt.float32, tag="spsum")
                nc.tensor.matmul(pb[:], ones_row[:], rec[:], start=True, stop=True)
                numer = sbf.tile([C, 512], f32, tag="numer")
                nc.vector.tensor_copy(numer[:], o_ps[0:C, :])
                out_t = sbf.tile([C, 512], f32, tag="out_t")
                nc.vector.tensor_mul(out_t[:], numer[:], pb[:])
                nc.sync.dma_start(o_d[:, mc * 512:(mc + 1) * 512], out_t[:])

    nc.compile()
    return nc


def _prep_in_maps(inputs):
    inp = {k: np.ascontiguousarray(np.asarray(v, dtype=np.float32))
           for k, v in inputs.items()}

    # structural assertions (guaranteed by the model constructor)
    for wname in ("wsx_vi", "wsy_vi", "wsx_ir", "wsy_ir", "wsx_q", "wsy_q"):
        w = inp[wname]
        assert np.all(w == w[0, 0]), f"{wname} is not a broadcast 3x3 kernel"
    Kx = inp["wsx_vi"][0, 0]
    Ky = inp["wsy_vi"][0, 0]
    assert np.array_equal(inp["wsx_q"][0, 0], Kx)
    assert np.array_equal(inp["wsy_q"][0, 0], Ky)
    assert np.array_equal(inp["wsx_ir"][0, 0], Kx)
    assert np.array_equal(inp["wsy_ir"][0, 0], Ky)

    alpha = {m: inp[f"w1_{m}"].sum(axis=1).astype(np.float32)
             for m in ("vi", "ir", "q")}
    b1q = inp["b1_q"]

    iden = np.eye(C, dtype=np.float32)
    ktap = np.broadcast_to(
        np.concatenate([Kx.ravel(), Ky.ravel()]).astype(np.float32)[None, :],
        (C, 18)).copy()
    csum = np.zeros((C + 1, 2), np.float32)
    csum[0:C, 0] = 1.0
    ones3 = np.ones((3, N), ml_dtypes.bfloat16)
    ones_r = np.ones((1, N), np.float32)

    def aug(x):
        return np.concatenate([x.reshape(C, N), ones_r], axis=0)

    def wv_aug_for(m):
        wa = np.zeros((C + 1, C + 1), np.float32)
        wa[0:C, 0:C] = inp[f"wv_{m}"].T
        wa[C, 0:C] = inp[f"bv_{m}"]
        wa[C, C] = 1.0       # ones column (denominator row)
        return wa

    xaug_b = {("vi", b): aug(inp["vi"][b]) for b in range(B)}
    xaug_b.update({("ir", b): aug(inp["ir"][b]) for b in range(B)})

    maps = []
    for core in range(8):
        b = core // 4
        vmod = "vi" if (core % 4) < 2 else "ir"
        kmod = "ir" if vmod == "vi" else "vi"
        half = core % 2
        ccv = np.zeros((C, 2), np.float32)
        ccv[:, 0] = np.float32(np.dot(alpha["q"], alpha[kmod]))
        ccv[:, 1] = np.float32(np.dot(b1q, alpha[kmod]))
        selm = np.zeros((C, 32), np.float32)
        for i in range(32):
            selm[half * 32 + i, i] = 1.0
        maps.append({
            "xaug": xaug_b[(vmod, b)],
            "xkaug": xaug_b[(kmod, b)],
            "wv_aug": wv_aug_for(vmod),
            "csum": csum,
            "iden": iden,
            "cc": ccv,
            "sel": selm,
            "ktap": ktap,
            "ones3": ones3,
        })
    return maps


def kernel(**inputs):
    from concourse.bass_utils import run_bass_kernel_spmd

    if "nc" not in _CACHE:
        _CACHE["nc"] = _build_program()
    nc = _CACHE["nc"]

    maps = _prep_in_maps(inputs)
    res = run_bass_kernel_spmd(nc, maps, list(range(8))).results

    vi_out = np.empty((B, C, H, W), np.float32)
    ir_out = np.empty((B, C, H, W), np.float32)
    for core in range(8):
        b = core // 4
        vmod = "vi" if (core % 4) < 2 else "ir"
        half = core % 2
        o = res[core]["o"].reshape(C, 32, W)
        dst = vi_out if vmod == "vi" else ir_out
        dst[b, :, half * 32:(half + 1) * 32, :] = o
    return vi_out, ir_out
